# revision 17
# baseline (speedup 1.0000x reference)
"""Trainium2 Bass kernel for nn_Net_16793322127774 (GNN message passing).

Data-parallel over B=256 graphs, 32 graphs per core on 8 cores.

v2 design (vs the hi/lo-split baseline):
  - Message passing as dense per-graph adjacency matmuls. The 256x256
    edge-count matrix is packed host-side as fp8e4 (small ints are exact),
    DMA'd once, resident in SBUF, reused by both layers.
  - Single-precision float32r matmuls everywhere (PSUM accumulates fp32);
    f32<->f32r moves are free AP bitcasts, not copies.
  - x is supplied in BOTH layouts from the host (node-major bf16 for the
    aggregation lhsT, feature-major bf16 for the root-weight rhs) packed so
    every DMA is contiguous per partition -- 16 DMA instructions total.
  - Per-graph scores accumulate into one [32,256] PSUM tile via
    block-column lhsT matmuls (p in column g); pooling = exact k-th value
    threshold via DVE Max8 chains (unchanged from baseline).
  - Readout max on the (otherwise idle) GPSIMD engine as an elementwise
    max tree; readout sums via ones-column matmuls on PE / one batched 3D
    DVE reduce; PSUM->SBUF staging on the Activation engine with fused
    bias+ReLU.
"""
import sys
sys.path.insert(0, "/opt/trn_rl_repo")
import numpy as np
import ml_dtypes
import concourse.bass as bass
import concourse.bacc as bacc
import concourse.mybir as mybir
from concourse.bass_utils import run_bass_kernel_spmd
from concourse.tile import TileContext
from concourse.masks import make_identity

P = 128
B, N, F, H, C = 256, 256, 128, 128, 6
NCORES = 8
GPC = B // NCORES            # 32 graphs per core
NPC = GPC * N                # 8192 nodes per core
K1, K2 = 205, 164
NEG = -1.0e30
CH = 4                       # graphs per compute chunk
NCH = GPC // CH
WV = 8                       # graphs per DMA wave
NWV = GPC // WV

f32 = mybir.dt.float32
f32r = mybir.dt.float32r
bf16 = mybir.dt.bfloat16
f8 = mybir.dt.float8e4
AX = mybir.AxisListType.X
OP = mybir.AluOpType
AF = mybir.ActivationFunctionType

# wcat packed-weight column layout
WC_W1REL, WC_W1ROOT, WC_W2REL, WC_W2ROOT = 0, 128, 256, 384
WC_WL1A, WC_WL1B, WC_WL2 = 512, 640, 768
WC_P1, WC_P2, WC_B1, WC_B2, WC_BL1, WC_BL2 = 774, 775, 776, 777, 778, 779
WC_COLS = 780

_cache = {}


def build_nc():
    nc = bacc.Bacc("TRN2", target_bir_lowering=False, debug=False)

    xT_in = nc.declare_dram_parameter("xT", [P, GPC, N], f32r, isOutput=False)
    xnm_in = nc.declare_dram_parameter("xnm", [P, GPC, 2, F], bf16, isOutput=False)
    at_in = nc.declare_dram_parameter("at8", [P, GPC, 2, N], f8, isOutput=False)
    wc_in = nc.declare_dram_parameter("wcat", [P, WC_COLS], f32r, isOutput=False)
    out_d = nc.declare_dram_parameter("out", [GPC, C], f32, isOutput=True)

    with TileContext(nc) as tc:
        with tc.tile_pool(name="pers", bufs=1) as pe, \
             tc.tile_pool(name="work", bufs=3) as wk, \
             tc.tile_pool(name="poolw", bufs=1) as pw, \
             tc.tile_pool(name="bigp", bufs=3, space="PSUM") as bigp, \
             tc.tile_pool(name="prp", bufs=1, space="PSUM") as prp:

            # ---------- setup ----------
            idf = pe.tile([P, P], f32)
            make_identity(nc, idf[:])
            idrt = pe.tile([P, P], f32r)
            nc.vector.tensor_copy(out=idrt[:], in_=idf[:])
            idr = idrt[:]

            wct = pe.tile([P, WC_COLS], f32r)
            nc.sync.dma_start(out=wct[:], in_=wc_in[:])

            onef = wk.tile([P, 1], f32, tag="onef")
            nc.vector.memset(onef[:], 1.0)
            ones1b = pe.tile([1, P], bf16)
            nc.vector.tensor_copy(out=ones1b[:],
                                  in_=onef[0:1, 0:1].broadcast_to((1, P)))
            onesb = pe.tile([P, 1], bf16)
            nc.vector.tensor_copy(out=onesb[:], in_=onef[:])

            p1blk = pe.tile([P, GPC, GPC], f32r)
            nc.vector.memset(p1blk[:].bitcast(f32), 0.0)
            p2blk = pe.tile([P, GPC, GPC], f32r)
            nc.vector.memset(p2blk[:].bitcast(f32), 0.0)
            for g in range(GPC):
                nc.vector.tensor_copy(out=p1blk[:, g, g:g + 1],
                                      in_=wct[:, WC_P1:WC_P1 + 1])
                nc.vector.tensor_copy(out=p2blk[:, g, g:g + 1],
                                      in_=wct[:, WC_P2:WC_P2 + 1])

            xTT = pe.tile([P, GPC, N], f32r)
            xnmT = pe.tile([P, GPC, 2, F], bf16)
            atT = pe.tile([P, GPC, 2, N], f8)
            h1T = pe.tile([P, NPC], f32r)
            h2T = pe.tile([P, NPC], f32r)

            waves = [(0, 4), (4, 4), (8, 8), (16, 8), (24, 8)]
            for w0, wn in waves:
                sl = slice(w0, w0 + wn)
                nc.sync.dma_start(out=atT[:, sl, :, :], in_=at_in[:, sl, :, :])
                nc.sync.dma_start(out=xnmT[:, sl, :, :], in_=xnm_in[:, sl, :, :])
                nc.sync.dma_start(out=xTT[:, sl, :], in_=xT_in[:, sl, :])

            w1rel_r = wct[:, WC_W1REL:WC_W1REL + H]
            w1root_r = wct[:, WC_W1ROOT:WC_W1ROOT + H]
            w2rel_r = wct[:, WC_W2REL:WC_W2REL + H]
            w2root_r = wct[:, WC_W2ROOT:WC_W2ROOT + H]
            wl1a_r = wct[:, WC_WL1A:WC_WL1A + H]
            wl1b_r = wct[:, WC_WL1B:WC_WL1B + H]
            wl2_r = wct[:, WC_WL2:WC_WL2 + C]
            b1ap = wct[:, WC_B1:WC_B1 + 1].bitcast(f32)
            b2ap = wct[:, WC_B2:WC_B2 + 1].bitcast(f32)
            bl1ap = wct[:, WC_BL1:WC_BL1 + 1].bitcast(f32)
            bl2ap = wct[0:C, WC_BL2:WC_BL2 + 1].bitcast(f32)

            # ---------- layer 1 ----------
            psS1 = prp.tile([GPC, N], f32, tag="psS")
            for ch in range(NCH):
                g0 = ch * CH
                psA = bigp.tile([P, CH * N], f32, tag="big")
                for k in range(CH):
                    g = g0 + k
                    for t in range(2):
                        nc.tensor.matmul(out=psA[:, k * N:(k + 1) * N],
                                         lhsT=xnmT[:, g, t, :],
                                         rhs=atT[:, g, t, :],
                                         start=(t == 0), stop=(t == 1))
                aggS = wk.tile([P, CH * N], f32r, tag="agg")
                nc.scalar.activation(out=aggS[:], in_=psA[:], func=AF.Copy)
                psH = bigp.tile([P, CH * N], f32, tag="big")
                for hh in range(2):
                    nc.tensor.matmul(out=psH[:, hh * 512:hh * 512 + 512],
                                     lhsT=w1rel_r,
                                     rhs=aggS[:, hh * 512:hh * 512 + 512],
                                     start=True, stop=False,
                                     skip_group_check=True)
                for hh in range(2):
                    nc.tensor.matmul(out=psH[:, hh * 512:hh * 512 + 512],
                                     lhsT=w1root_r,
                                     rhs=xTT[:, g0 + 2 * hh:g0 + 2 * hh + 2, :],
                                     start=False, stop=True,
                                     skip_group_check=True)
                nc.scalar.activation(out=h1T[:, g0 * N:(g0 + CH) * N],
                                     in_=psH[:], func=AF.Relu, bias=b1ap)
                # scores for this chunk (accumulates into psS1 across chunks)
                for k in range(CH):
                    g = g0 + k
                    nc.tensor.matmul(out=psS1[:], lhsT=p1blk[:, g, :],
                                     rhs=h1T[:, g * N:(g + 1) * N],
                                     start=(g == 0), stop=(g == GPC - 1),
                                     skip_group_check=True)
            S1 = pe.tile([GPC, N], f32)
            nc.scalar.activation(out=S1[:], in_=psS1[:], func=AF.Copy)
            V0e = None

            # ---------- pool 1 ----------
            V0 = pw.tile([GPC, N], f32, tag="V0")
            V1 = pw.tile([GPC, N], f32, tag="V1")
            nc.scalar.activation(out=V0[:], in_=psS1[:], func=AF.Copy, scale=-1.0)
            vs = [V0, V1]
            mx = None
            for r in range(7):
                mx = pw.tile([GPC, 8], f32, tag=f"mx1_{r}")
                nc.vector.max(out=mx[:], in_=vs[r % 2][:])
                if r < 6:
                    nc.vector.match_replace(out=vs[(r + 1) % 2][:], in_to_replace=mx[:],
                                            in_values=vs[r % 2][:], imm_value=NEG)
            thr1 = pw.tile([GPC, 1], f32, tag="thr1")
            nc.vector.tensor_scalar(out=thr1[:], in0=mx[:, 3:4], scalar1=-1.0,
                                    scalar2=None, op0=OP.mult)
            kept1 = pe.tile([GPC, N], f32)
            nc.vector.tensor_scalar(out=kept1[:], in0=S1[:], scalar1=thr1[:],
                                    scalar2=None, op0=OP.is_ge)
            T1 = pw.tile([GPC, N], f32, tag="T1")
            nc.scalar.activation(out=T1[:], in_=S1[:], func=AF.Tanh)
            M1 = pw.tile([GPC, N], bf16, tag="M1")
            nc.vector.tensor_tensor(out=M1[:], in0=T1[:], in1=kept1[:], op=OP.mult)

            # ---------- scale h1 -> g1 (in place) ----------
            mflat = pe.tile([1, NPC], bf16)
            nc.sync.dma_start(out=mflat[0:1, :], in_=M1[:, :])
            for c in range(16):
                cs = c * 512
                psM = bigp.tile([P, 512], f32, tag="big")
                nc.tensor.matmul(out=psM[:], lhsT=ones1b[:],
                                 rhs=mflat[0:1, cs:cs + 512],
                                 start=True, stop=True)
                nc.vector.tensor_tensor(out=h1T[:, cs:cs + 512],
                                        in0=h1T[:, cs:cs + 512],
                                        in1=psM[:], op=OP.mult)

            # readout 1 max (overlaps layer 2 on DVE)
            h1v = h1T[:].rearrange("p (g n) -> p g n", g=GPC)
            Xm1 = pe.tile([P, GPC], f32)
            nc.vector.tensor_reduce(out=Xm1[:], in_=h1v, axis=AX, op=OP.max)

            # ---------- layer 2 ----------
            psXs = prp.tile([P, GPC], f32, tag="psX")
            psS2 = prp.tile([GPC, N], f32, tag="psS")
            for ch in range(NCH):
                g0 = ch * CH
                psT = bigp.tile([P, CH * N], f32, tag="big")
                for k in range(CH):
                    g = g0 + k
                    for t in range(2):
                        nc.tensor.matmul(
                            out=psT[:, (2 * k + t) * F:(2 * k + t + 1) * F],
                            lhsT=h1T[:, g * N + t * F:g * N + (t + 1) * F],
                            rhs=idr[0:P, 0:P], start=True, stop=True)
                gnS = wk.tile([P, CH * N], bf16, tag="gn")
                nc.scalar.activation(out=gnS[:], in_=psT[:], func=AF.Copy)
                psA = bigp.tile([P, CH * N], f32, tag="big")
                for k in range(CH):
                    g = g0 + k
                    for t in range(2):
                        # agg + node-sum share the same stationary weights
                        lw = gnS[:, (2 * k + t) * F:(2 * k + t + 1) * F]
                        nc.tensor.matmul(out=psA[:, k * N:(k + 1) * N],
                                         lhsT=lw, rhs=atT[:, g, t, :],
                                         start=(t == 0), stop=(t == 1),
                                         skip_group_check=True)
                        nc.tensor.matmul(out=psXs[:, g:g + 1],
                                         lhsT=lw, rhs=onesb[:],
                                         start=(t == 0), stop=(t == 1),
                                         skip_group_check=True)
                aggS = wk.tile([P, CH * N], f32r, tag="agg")
                nc.scalar.activation(out=aggS[:], in_=psA[:], func=AF.Copy)
                psH = bigp.tile([P, CH * N], f32, tag="big")
                for hh in range(2):
                    nc.tensor.matmul(out=psH[:, hh * 512:hh * 512 + 512],
                                     lhsT=w2rel_r,
                                     rhs=aggS[:, hh * 512:hh * 512 + 512],
                                     start=True, stop=False,
                                     skip_group_check=True)
                for hh in range(2):
                    cs = hh * 512
                    nc.tensor.matmul(out=psH[:, cs:cs + 512], lhsT=w2root_r,
                                     rhs=h1T[:, g0 * N + cs:g0 * N + cs + 512],
                                     start=False, stop=True,
                                     skip_group_check=True)
                nc.scalar.activation(out=h2T[:, g0 * N:(g0 + CH) * N],
                                     in_=psH[:], func=AF.Relu, bias=b2ap)
                for k in range(CH):
                    g = g0 + k
                    nc.tensor.matmul(out=psS2[:], lhsT=p2blk[:, g, :],
                                     rhs=h2T[:, g * N:(g + 1) * N],
                                     start=(g == 0), stop=(g == GPC - 1),
                                     skip_group_check=True)
            S2 = pe.tile([GPC, N], f32)
            nc.scalar.activation(out=S2[:], in_=psS2[:], func=AF.Copy)

            # ---------- pool 2 ----------
            n2a = pw.tile([GPC, N], f32, tag="n2a")
            nc.vector.tensor_scalar(out=n2a[:], in0=S2[:], scalar1=-1.0,
                                    scalar2=None, op0=OP.mult)
            W0 = pw.tile([GPC, N], f32, tag="W0")
            W1t = pw.tile([GPC, N], f32, tag="W1t")
            t1m = pw.tile([GPC, N], f32, tag="t1m")
            nc.vector.tensor_tensor(out=t1m[:], in0=n2a[:], in1=kept1[:], op=OP.mult)
            um = pw.tile([GPC, N], f32, tag="um")
            nc.vector.tensor_scalar(out=um[:], in0=kept1[:], scalar1=1.0, scalar2=-NEG,
                                    op0=OP.subtract, op1=OP.mult)
            nc.vector.tensor_tensor(out=W0[:], in0=t1m[:], in1=um[:], op=OP.add)
            ws = [W0, W1t]
            mx2 = None
            for r in range(6):
                mx2 = pw.tile([GPC, 8], f32, tag=f"mx2_{r}")
                nc.vector.max(out=mx2[:], in_=ws[r % 2][:])
                if r < 5:
                    nc.vector.match_replace(out=ws[(r + 1) % 2][:], in_to_replace=mx2[:],
                                            in_values=ws[r % 2][:], imm_value=NEG)
            thr2 = pw.tile([GPC, 1], f32, tag="thr2")
            nc.vector.tensor_scalar(out=thr2[:], in0=mx2[:, 1:2], scalar1=-1.0,
                                    scalar2=None, op0=OP.mult)
            kge = pw.tile([GPC, N], f32, tag="kge")
            nc.vector.tensor_scalar(out=kge[:], in0=S2[:], scalar1=thr2[:],
                                    scalar2=None, op0=OP.is_ge)
            kept2 = pw.tile([GPC, N], f32, tag="kept2")
            nc.vector.tensor_tensor(out=kept2[:], in0=kge[:], in1=kept1[:], op=OP.mult)
            T2 = pw.tile([GPC, N], f32, tag="T2")
            nc.scalar.activation(out=T2[:], in_=S2[:], func=AF.Tanh)
            M2 = pw.tile([GPC, N], bf16, tag="M2")
            nc.vector.tensor_tensor(out=M2[:], in0=T2[:], in1=kept2[:], op=OP.mult)

            # ---------- scale h2 -> g2 (in place) + readout 2, pipelined ----------
            nc.sync.dma_start(out=mflat[0:1, :], in_=M2[:, :])
            h2v = h2T[:].rearrange("p (g n) -> p g n", g=GPC)
            Xs2 = pe.tile([P, GPC], f32)
            Xm2 = pe.tile([P, GPC], f32)
            for q in range(4):          # 8 graphs per quarter
                for c in range(4 * q, 4 * q + 4):
                    cs = c * 512
                    psM = bigp.tile([P, 512], f32, tag="big")
                    nc.tensor.matmul(out=psM[:], lhsT=ones1b[:],
                                     rhs=mflat[0:1, cs:cs + 512],
                                     start=True, stop=True)
                    nc.vector.tensor_tensor(out=h2T[:, cs:cs + 512],
                                            in0=h2T[:, cs:cs + 512],
                                            in1=psM[:], op=OP.mult)
                gq = slice(q * 8, q * 8 + 8)
                nc.vector.tensor_reduce(out=Xs2[:, gq], in_=h2v[:, gq, :],
                                        axis=AX, op=OP.add)
                nc.vector.tensor_reduce(out=Xm2[:, gq], in_=h2v[:, gq, :],
                                        axis=AX, op=OP.max)

            # ---------- head (feature-major) ----------
            zA = wk.tile([P, GPC], f32r, tag="zA")
            nc.vector.tensor_tensor(out=zA[:], in0=Xm1[:], in1=Xm2[:], op=OP.add)
            t2s = wk.tile([P, GPC], f32, tag="t2s")
            nc.vector.tensor_scalar(out=t2s[:], in0=Xs2[:], scalar1=1.0 / K2,
                                    scalar2=None, op0=OP.mult)
            zB = wk.tile([P, GPC], f32r, tag="zB")
            nc.vector.scalar_tensor_tensor(out=zB[:], in0=psXs[:], scalar=1.0 / K1,
                                           in1=t2s[:], op0=OP.mult, op1=OP.add)

            psZ = bigp.tile([P, GPC], f32, tag="big")
            nc.tensor.matmul(out=psZ[:], lhsT=wl1a_r, rhs=zA[:],
                             start=True, stop=False)
            nc.tensor.matmul(out=psZ[:], lhsT=wl1b_r, rhs=zB[:],
                             start=False, stop=True)
            z2 = wk.tile([P, GPC], f32r, tag="z2")
            nc.scalar.activation(out=z2[:], in_=psZ[:], func=AF.Relu, bias=bl1ap)

            psO = bigp.tile([C, GPC], f32, tag="big")
            nc.tensor.matmul(out=psO[:], lhsT=wl2_r, rhs=z2[:],
                             start=True, stop=True)
            oT = wk.tile([C, GPC], f32r, tag="oT")
            nc.scalar.activation(out=oT[:], in_=psO[:], func=AF.Identity, bias=bl2ap)

            psZo = bigp.tile([GPC, C], f32, tag="big")
            nc.tensor.matmul(out=psZo[:], lhsT=oT[:],
                             rhs=idr[0:C, 0:C], start=True, stop=True)
            zo = wk.tile([GPC, C], f32, tag="zo")
            nc.vector.tensor_copy(out=zo[:], in_=psZo[:])

            mxo = pw.tile([GPC, 1], f32, tag="mxo")
            nc.vector.tensor_reduce(out=mxo[:], in_=zo[:], axis=AX, op=OP.max)
            nmx = pw.tile([GPC, 1], f32, tag="nmx")
            nc.vector.tensor_scalar(out=nmx[:], in0=mxo[:], scalar1=-1.0,
                                    scalar2=None, op0=OP.mult)
            ex = pw.tile([GPC, C], f32, tag="ex")
            se = pw.tile([GPC, 1], f32, tag="se")
            nc.scalar.activation(out=ex[:], in_=zo[:], func=AF.Exp, bias=nmx[:],
                                 accum_out=se[:])
            lnse = pw.tile([GPC, 1], f32, tag="lnse")
            nc.scalar.activation(out=lnse[:], in_=se[:], func=AF.Ln)
            o1 = pw.tile([GPC, C], f32, tag="o1")
            nc.vector.tensor_scalar(out=o1[:], in0=zo[:], scalar1=mxo[:],
                                    scalar2=None, op0=OP.subtract)
            o2 = pw.tile([GPC, C], f32, tag="o2")
            nc.vector.tensor_scalar(out=o2[:], in0=o1[:], scalar1=lnse[:],
                                    scalar2=None, op0=OP.subtract)
            nc.sync.dma_start(out=out_d[:], in_=o2[:])

    nc.compile()
    return nc


def kernel(**inputs):
    x = np.asarray(inputs["x"], np.float32)
    src = np.asarray(inputs["src"], np.int64)
    dst = np.asarray(inputs["dst"], np.int64)

    if "nc" not in _cache:
        _cache["nc"] = build_nc()
    nc = _cache["nc"]

    wcat = np.zeros((P, WC_COLS), np.float32)
    wcat[:, WC_W1REL:WC_W1REL + H] = np.asarray(inputs["W1_rel"], np.float32).T
    wcat[:, WC_W1ROOT:WC_W1ROOT + H] = np.asarray(inputs["W1_root"], np.float32).T
    wcat[:, WC_W2REL:WC_W2REL + H] = np.asarray(inputs["W2_rel"], np.float32).T
    wcat[:, WC_W2ROOT:WC_W2ROOT + H] = np.asarray(inputs["W2_root"], np.float32).T
    wl1T = np.asarray(inputs["W_lin1"], np.float32).T          # [2H, H]
    wcat[:, WC_WL1A:WC_WL1A + H] = wl1T[0:H, :]
    wcat[:, WC_WL1B:WC_WL1B + H] = wl1T[H:2 * H, :]
    wcat[:, WC_WL2:WC_WL2 + C] = np.asarray(inputs["W_lin2"], np.float32).T
    p1 = np.asarray(inputs["p1"], np.float32)
    p2 = np.asarray(inputs["p2"], np.float32)
    wcat[:, WC_P1] = p1 / np.float32(np.linalg.norm(p1))
    wcat[:, WC_P2] = p2 / np.float32(np.linalg.norm(p2))
    wcat[:, WC_B1] = np.asarray(inputs["b1_rel"], np.float32)
    wcat[:, WC_B2] = np.asarray(inputs["b2_rel"], np.float32)
    wcat[:, WC_BL1] = np.asarray(inputs["b_lin1"], np.float32)
    wcat[0:C, WC_BL2] = np.asarray(inputs["b_lin2"], np.float32)

    epc = src.shape[0] // NCORES
    in_maps = []
    for c in range(NCORES):
        xl = x[c * NPC:(c + 1) * NPC]
        xTl = np.ascontiguousarray(xl.T.reshape(P, GPC, N))
        xnm = np.ascontiguousarray(
            xl.reshape(GPC, 2, P, F).transpose(2, 0, 1, 3)).astype(ml_dtypes.bfloat16)
        s = src[c * epc:(c + 1) * epc] - c * NPC
        d = dst[c * epc:(c + 1) * epc] - c * NPC
        cell = s * N + (d % N)
        A = np.bincount(cell, minlength=GPC * N * N).reshape(GPC, N, N)
        at8 = np.ascontiguousarray(
            A.reshape(GPC, 2, P, N).transpose(2, 0, 1, 3)).astype(ml_dtypes.float8_e4m3)
        in_maps.append(dict(xT=xTl, xnm=xnm, at8=at8, wcat=wcat))

    _cache["last_in_maps"] = in_maps
    res = run_bass_kernel_spmd(nc, in_maps, list(range(NCORES)))
    _cache["last_res"] = res
    return np.concatenate([r["out"] for r in res.results], axis=0)


def __getattr__(name):
    if name == "_last_in_maps":
        return _cache["last_in_maps"]
    raise AttributeError(name)
